# revision 13
# baseline (speedup 1.0000x reference)
"""Trainium2 Bass kernel v2 for the GCN message-passing model (8 NeuronCores).

Key changes vs v1:
- Balanced dst->block assignment (NBLK=102) cuts gather padding ~18%->~2%.
- LayerNorm stats are deferred: the gathered quantity is z = dinv*(y@W)
  (computable before the stats AllReduce); the LN affine correction is
  applied in the next epilogue via the static per-dst sum S[d] and a rank-1
  (Wbar x SS) term.  This removes the stats AllReduce from the critical path.
- The hws AllGather is split into 4 piece collectives; window-j gathers are
  gated only on collective j, so collectives overlap the gather phase.
- Chunk-local psum segments + batched is_equal seg build per chunk.
"""

import os
import sys

sys.path.insert(0, "/opt/trn_rl_repo")

import numpy as np
import ml_dtypes

import concourse.bass as bass
import concourse.bacc as bacc
import concourse.mybir as mybir
import concourse.tile as tile
from concourse.bass_utils import run_bass_kernel_spmd
from concourse.library_config import mlp as gpsimd_mlp_lib
from concourse.masks import make_identity

NCORES = 8
N_NODES = 100_000
F = 128
NCLS = 10
LAYERS = 3
NGRAPH = 256
EPS = 1e-5
NSH = N_NODES // NCORES            # 12500 real nodes per core
NBLK = 102                         # dst blocks of 128 slots per core
SLOTS = NBLK * 128                 # 13056
NPIECE = 4
PIECE_BLKS = [26, 26, 25, 25]
PIECE_CAP = [pb * 128 for pb in PIECE_BLKS]          # 3328,3328,3200,3200
PIECE_BSTART = [0, 26, 52, 77]
PIECE_SSTART = [0, 3328, 6656, 9856]
PIECE_REAL = NSH // NPIECE                            # 3125 real nodes/piece
CHUNK = 1024
TPC = CHUNK // 128
NQUEUE = 4
SLABCH = 16

BF16 = mybir.dt.bfloat16
F32 = mybir.dt.float32
I16 = mybir.dt.int16
AOP = mybir.AluOpType
AF = mybir.ActivationFunctionType
BF = ml_dtypes.bfloat16


def _balance(dw):
    """Assign nodes to blocks per (core, piece), balancing per-window loads.

    dw: [NCORES, PIECE_REAL, 4] per-node window in-degree vectors for one
    piece across all cores.  Returns block index [NCORES, PIECE_REAL] in
    [0, nb) for the given piece size nb."""
    nc_, n, _ = dw.shape

    def run(nb):
        order = np.argsort(-dw.sum(axis=2), axis=1, kind="stable")
        loads = np.zeros((nc_, nb, 4), np.float64)
        sizes = np.zeros((nc_, nb), np.int64)
        assign = np.empty((nc_, n), np.int64)
        ar = np.arange(nc_)
        for r in range(n):
            i = order[:, r]
            dr = dw[ar, i]                       # [nc, 4]
            cand = (loads + dr[:, None, :]).max(axis=2)   # [nc, nb]
            cand[sizes >= 128] = np.inf
            b = np.argmin(cand, axis=1)
            loads[ar, b] += dr
            sizes[ar, b] += 1
            assign[ar, i] = b
        return assign

    return run


def _host_preprocess(edge_index):
    # self-loops are handled locally on-device (z^T acc init), so the edge
    # stream here contains only the real edges; deg/dinv/S keep the loops.
    src = np.asarray(edge_index[0], dtype=np.int64)
    dst = np.asarray(edge_index[1], dtype=np.int64)
    loop = np.arange(N_NODES, dtype=np.int64)
    src_l = np.concatenate([src, loop])
    dst_l = np.concatenate([dst, loop])
    deg = np.bincount(dst_l, minlength=N_NODES).astype(np.float64)
    dinv = (1.0 / np.sqrt(deg)).astype(np.float32)
    S = np.bincount(dst_l, weights=dinv[src_l].astype(np.float64),
                    minlength=N_NODES).astype(np.float32)

    owner = np.arange(N_NODES) // NSH
    ln = np.arange(N_NODES) % NSH
    piece = ln // PIECE_REAL                       # window of this node
    win_e = piece[src]                             # per-edge window

    # per-node window in-degree vectors
    dwin = np.bincount(dst * 4 + win_e, minlength=N_NODES * 4) \
        .reshape(N_NODES, 4).astype(np.float64)

    # balanced block assignment per (core, piece)
    blk_of = np.empty(N_NODES, np.int64)
    for p in range(NPIECE):
        ids = (np.arange(NCORES)[:, None] * NSH + p * PIECE_REAL
               + np.arange(PIECE_REAL)[None, :])          # [8, 3125]
        runner = _balance(dwin[ids])
        assign = runner(PIECE_BLKS[p])                    # [8, 3125]
        blk_of[ids] = PIECE_BSTART[p] + assign

    # repair pass: swap nodes between blocks (same core+piece) until no
    # (core, block, window) cell exceeds 8 tiles (1024 edges)
    pstart = np.array(PIECE_BSTART + [NBLK])
    cnt = np.zeros((NCORES, NBLK, 4), np.int64)
    np.add.at(cnt, (owner, blk_of, piece), dwin.astype(np.int64))
    cap = 8 * 128
    for _ in range(6000):
        over = cnt - cap
        c, b, w = np.unravel_index(np.argmax(over), over.shape)
        if over[c, b, w] <= 0:
            break
        p = int(np.searchsorted(pstart, b, side="right")) - 1
        blo, bhi = pstart[p], pstart[p + 1]
        b2 = blo + int(np.argmin(cnt[c, blo:bhi, w]))
        # node in (c,b) with largest w-degree; partner in (c,b2) with least
        in_b = np.where((owner == c) & (blk_of == b))[0]
        in_b2 = np.where((owner == c) & (blk_of == b2))[0]
        i = in_b[np.argmax(dwin[in_b, w])]
        blk_of[i] = b2
        cnt[c, b] -= dwin[i].astype(np.int64)
        cnt[c, b2] += dwin[i].astype(np.int64)
        if len(in_b2) >= 128:
            j = in_b2[np.argmin(dwin[in_b2, w])]
            blk_of[j] = b
            cnt[c, b2] -= dwin[j].astype(np.int64)
            cnt[c, b] += dwin[j].astype(np.int64)

    # slot within block: order by node id within each (core, block)
    order = np.lexsort((np.arange(N_NODES), blk_of + owner * NBLK))
    key_cb = (owner * NBLK + blk_of)[order]
    slotin = np.empty(N_NODES, np.int64)
    # running index within equal key runs
    change = np.empty(len(key_cb), bool)
    change[0] = True
    change[1:] = key_cb[1:] != key_cb[:-1]
    runstart = np.maximum.accumulate(np.where(change, np.arange(len(key_cb)), 0))
    slotin[order] = np.arange(len(key_cb)) - runstart
    assert slotin.max() < 128
    ls_of = blk_of * 128 + slotin                  # local slot in [0, SLOTS)

    # gather index within the source's window table
    cap_w = np.array(PIECE_CAP)[piece]
    sstart_w = np.array(PIECE_SSTART)[piece]
    widx = owner * cap_w + (ls_of - sstart_w)
    assert widx.min() >= 0 and (widx < 8 * cap_w).all()

    # group edges by (core, block, window)
    key = (owner[dst] * NBLK + blk_of[dst]) * 4 + win_e
    eorder = np.argsort(key, kind="stable")
    key_s = key[eorder]
    ngroups = NCORES * NBLK * 4
    counts = np.bincount(key_s, minlength=ngroups).reshape(NCORES, NBLK, 4)
    starts = np.zeros(ngroups + 1, dtype=np.int64)
    np.cumsum(counts.reshape(-1), out=starts[1:])
    gidx_s = widx[src[eorder]]
    slot_s = slotin[dst[eorder]]

    T = (counts.max(axis=0) + 127) // 128          # [NBLK, 4]
    for w in range(NPIECE):
        T[NBLK - 1, w] += (-int(T[:, w].sum())) % TPC
    TT = int(T.sum())
    ecap = TT * 128

    tile_block = np.empty(TT, dtype=np.int64)
    tile_window = np.empty(TT, dtype=np.int64)
    t0 = 0
    for w in range(NPIECE):
        for b in range(NBLK):
            n = int(T[b, w])
            tile_block[t0:t0 + n] = b
            tile_window[t0:t0 + n] = w
            t0 += n
    assert t0 == TT

    idx16 = np.zeros((NCORES, ecap), dtype=np.int16)
    slots = np.full((NCORES, ecap), 255, dtype=np.float32)
    for c in range(NCORES):
        pos = 0
        for w in range(NPIECE):
            for b in range(NBLK):
                g = (c * NBLK + b) * 4 + w
                s0, s1 = starts[g], starts[g + 1]
                n = s1 - s0
                idx16[c, pos:pos + n] = gidx_s[s0:s1].astype(np.int16)
                slots[c, pos:pos + n] = slot_s[s0:s1].astype(np.float32)
                pos += int(T[b, w]) * 128
        assert pos == ecap

    idxw = idx16.reshape(NCORES, -1, 16).transpose(0, 2, 1)
    idxw = np.ascontiguousarray(np.tile(idxw, (1, 8, 1)))
    slotw = np.ascontiguousarray(
        slots.reshape(NCORES, TT, 128).transpose(0, 2, 1)).astype(BF)

    meta = dict(TT=TT, tile_block=tile_block, tile_window=tile_window)
    return dinv, S, blk_of, slotin, ls_of, idxw, slotw, meta


def _wrap_cols(vec, fill, dtype=np.float32):
    """[SLOTS] -> [128, NBLK] with slot b*128+p at [p, b]."""
    return np.ascontiguousarray(vec.reshape(NBLK, 128).T).astype(dtype)


def _build_program(meta, cut="full"):
    TT = meta["TT"]
    tile_block = meta["tile_block"]
    tile_window = meta["tile_window"]
    ECAP = TT * 128
    # chunks per window
    wtiles = [int((tile_window == w).sum()) for w in range(NPIECE)]
    assert all(t % TPC == 0 for t in wtiles)
    NCHUNK = ECAP // CHUNK

    nc = bacc.Bacc("TRN2", target_bir_lowering=False, debug=False,
                   num_devices=NCORES, num_swdge_queues=NQUEUE)

    xT_in = nc.declare_dram_parameter("xT", [F, SLOTS], BF16, isOutput=False)
    xTf_in = nc.declare_dram_parameter("xTf", [F, NCORES * SLOTS], BF16,
                                       isOutput=False)
    dinvwf_in = nc.declare_dram_parameter("dinvwf", [128, NCORES * NBLK], F32,
                                          isOutput=False)
    idx_in = nc.declare_dram_parameter("idx", [128, ECAP // 16], I16, isOutput=False)
    slot_in = nc.declare_dram_parameter("slot", [128, TT], BF16, isOutput=False)
    dinvrep_in = nc.declare_dram_parameter("dinvrep", [128, SLOTS], BF16, isOutput=False)
    ssrep_in = nc.declare_dram_parameter("ssrep", [128, SLOTS], BF16, isOutput=False)
    dinvw_in = nc.declare_dram_parameter("dinvw", [128, NBLK], F32, isOutput=False)
    pslot_in = nc.declare_dram_parameter("pslot", [128, NBLK], BF16, isOutput=False)
    iota128_in = nc.declare_dram_parameter("iota128", [128, 128], BF16, isOutput=False)
    iota256_in = nc.declare_dram_parameter("iota256", [128, 256], BF16, isOutput=False)
    lin1W_in = nc.declare_dram_parameter("lin1W", [F, F], BF16, isOutput=False)
    lin1b_in = nc.declare_dram_parameter("lin1b", [F, 1], F32, isOutput=False)
    convW_in = nc.declare_dram_parameter("convW", [F, LAYERS * F], BF16, isOutput=False)
    convb_in = nc.declare_dram_parameter("convb", [F, LAYERS], F32, isOutput=False)
    wbar_in = nc.declare_dram_parameter("wbar", [F, LAYERS], F32, isOutput=False)
    mlpW1_in = nc.declare_dram_parameter("mlpW1", [F, F], BF16, isOutput=False)
    mlpb1_in = nc.declare_dram_parameter("mlpb1", [F, 1], F32, isOutput=False)
    mlpW2_in = nc.declare_dram_parameter("mlpW2", [F, NCLS], BF16, isOutput=False)
    mlpb2r_in = nc.declare_dram_parameter("mlpb2r", [128, NCLS], F32, isOutput=False)
    invcntr_in = nc.declare_dram_parameter("invcntr", [128, NGRAPH], F32, isOutput=False)
    out_ext = nc.declare_dram_parameter("out", [NGRAPH, NCLS], F32, isOutput=True)

    rg = [list(range(NCORES))]
    NSTRIP = 6
    STRW = SLOTS // NSTRIP          # 2176 cols per strip
    assert STRW * NSTRIP == SLOTS

    with tile.TileContext(nc) as tc:
        with tc.tile_pool(name="const", bufs=1) as cst, \
             tc.tile_pool(name="big", bufs=1) as big, \
             tc.tile_pool(name="work", bufs=4) as work, \
             tc.tile_pool(name="esc", bufs=2) as escp, \
             tc.tile_pool(name="gbuf", bufs=6) as gpool, \
             tc.tile_pool(name="idxs", bufs=3) as ipool, \
             tc.tile_pool(name="seg", bufs=4) as segp, \
             tc.tile_pool(name="psum", bufs=4, space="PSUM") as pp, \
             tc.tile_pool(name="ppool", bufs=1, space="PSUM") as ppool, \
             tc.tile_pool(name="dram", bufs=1, space="DRAM") as dram, \
             tc.tile_pool(name="dram2", bufs=1, space="DRAM") as dram2:

            nc.gpsimd.load_library(gpsimd_mlp_lib)

            # ---- persistent SBUF constants ----
            slot_t = cst.tile([128, TT], BF16)
            nc.sync.dma_start(out=slot_t[:], in_=slot_in[:])
            dinvrep = cst.tile([128, SLOTS], BF16)
            nc.sync.dma_start(out=dinvrep[:], in_=dinvrep_in[:])
            ssrep = cst.tile([128, SLOTS], BF16)
            nc.sync.dma_start(out=ssrep[:], in_=ssrep_in[:])
            dinvw = cst.tile([128, NBLK], F32)
            nc.sync.dma_start(out=dinvw[:], in_=dinvw_in[:])
            dinvwf = cst.tile([128, NCORES * NBLK], F32)
            nc.sync.dma_start(out=dinvwf[:], in_=dinvwf_in[:])
            pslot = cst.tile([128, NBLK], BF16)
            nc.sync.dma_start(out=pslot[:], in_=pslot_in[:])
            iota128 = cst.tile([128, 128], BF16)
            nc.sync.dma_start(out=iota128[:], in_=iota128_in[:])
            iota256 = cst.tile([128, 256], BF16)
            nc.sync.dma_start(out=iota256[:], in_=iota256_in[:])
            lin1W = cst.tile([F, F], BF16)
            nc.sync.dma_start(out=lin1W[:], in_=lin1W_in[:])
            lin1b = cst.tile([F, 1], F32)
            nc.sync.dma_start(out=lin1b[:], in_=lin1b_in[:])
            convW = cst.tile([F, LAYERS * F], BF16)
            nc.sync.dma_start(out=convW[:], in_=convW_in[:])
            convb = cst.tile([F, LAYERS], F32)
            nc.sync.dma_start(out=convb[:], in_=convb_in[:])
            wbar = cst.tile([F, LAYERS], F32)
            nc.sync.dma_start(out=wbar[:], in_=wbar_in[:])
            ones_col = cst.tile([128, 1], BF16)
            nc.vector.memset(ones_col[:], 1.0)
            ones_row1 = cst.tile([1, 128], BF16)
            nc.vector.memset(ones_row1[:], 1.0)
            ident = cst.tile([128, 128], BF16)
            make_identity(nc, ident[:])

            acc = big.tile([128, SLOTS], F32)

            for _ in range(6):
                g0 = gpool.tile([128, TPC, F], BF16, tag="gb")
                nc.vector.memset(g0[:].rearrange("p a b -> p (a b)"), 0.0)

            # DRAM staging: z pieces + double-buffered gather tables
            zin = [dram.tile([PIECE_CAP[j], F], BF16, tag=f"zin{j}")
                   for j in range(NPIECE)]
            tab = [[dram.tile([NCORES * PIECE_CAP[j], F], BF16,
                              tag=f"tab{par}_{j}")
                    for j in range(NPIECE)] for par in range(2)]

            # ---------------- helpers ----------------
            def emit_pass2(li, get_y16):
                """Produce z_{li+1} pieces + their collectives.  li=-1 is P0."""
                parity = (li + 1) % 2
                W = convW[:, (li + 1) * F:(li + 2) * F]
                for j in range(NPIECE):
                    for bl in range(PIECE_BLKS[j]):
                        b = PIECE_BSTART[j] + bl
                        y16 = get_y16(b)
                        ps = pp.tile([128, F], F32, tag="mm")
                        nc.tensor.matmul(out=ps[:], lhsT=y16, rhs=W,
                                         start=True, stop=True)
                        hb = work.tile([128, F], BF16, tag="hb")
                        nc.vector.tensor_scalar(
                            out=hb[:], in0=ps[:],
                            scalar1=dinvw[:, b:b + 1], scalar2=None,
                            op0=AOP.mult)
                        nc.sync.dma_start(out=zin_ap(j, bl), in_=hb[:])
                        psT = pp.tile([128, 128], F32, tag="mm")
                        nc.tensor.matmul(out=psT[:], lhsT=W, rhs=y16,
                                         start=True, stop=True)
                        nc.vector.tensor_tensor(
                            out=acc[:, b * 128:(b + 1) * 128], in0=psT[:],
                            in1=dinvrep[:, b * 128:(b + 1) * 128],
                            op=AOP.mult)
                    if j % 2 == 1:
                        nc.gpsimd.collective_compute(
                            "AllGather", AOP.bypass, replica_groups=rg,
                            ins=[zin[j // 2][:]],
                            outs=[tab[parity][j // 2][:]])

            # ---------------- P0 ----------------
            def p0_y16(b):
                xb = work.tile([128, 128], BF16, tag="xb")
                nc.sync.dma_start(out=xb[:],
                                  in_=xT_in[:, b * 128:(b + 1) * 128])
                ps = pp.tile([128, 128], F32, tag="mm")
                nc.tensor.matmul(out=ps[:], lhsT=lin1W[:], rhs=xb[:],
                                 start=True, stop=True)
                y = work.tile([128, 128], BF16, tag="y16")
                nc.scalar.activation(out=y[:], in_=ps[:], func=AF.Relu,
                                     bias=lin1b[:], scale=1.0)
                return y[:]

            # ---- P0: local full-table z1 (no collectives needed) ----
            W1 = convW[:, 0:F]
            WROWS0 = NCORES * PIECE_CAP[0]
            for j in range(NPIECE):
                for cc in range(NCORES):
                    for bl in range(PIECE_BLKS[j]):
                        gblk = cc * NBLK + PIECE_BSTART[j] + bl
                        xb = work.tile([128, 128], BF16, tag="xb")
                        nc.sync.dma_start(
                            out=xb[:],
                            in_=xTf_in[:, gblk * 128:(gblk + 1) * 128])
                        ps = pp.tile([128, 128], F32, tag="mm")
                        nc.tensor.matmul(out=ps[:], lhsT=lin1W[:], rhs=xb[:],
                                         start=True, stop=True)
                        y = work.tile([128, 128], BF16, tag="y16")
                        nc.scalar.activation(out=y[:], in_=ps[:],
                                             func=AF.Relu,
                                             bias=lin1b[:], scale=1.0)
                        ps2 = pp.tile([128, F], F32, tag="mm")
                        nc.tensor.matmul(out=ps2[:], lhsT=y[:], rhs=W1,
                                         start=True, stop=True)
                        hb = work.tile([128, F], BF16, tag="hb")
                        nc.vector.tensor_scalar(
                            out=hb[:], in0=ps2[:],
                            scalar1=dinvwf[:, gblk:gblk + 1], scalar2=None,
                            op0=AOP.mult)
                        roff = ((j % 2) * WROWS0 + cc * PIECE_CAP[0]
                                + bl * 128)
                        nc.sync.dma_start(
                            out=tab[0][j // 2][roff:roff + 128, :],
                            in_=hb[:])
            # own-shard z1^T acc init (self-loop term)
            for b in range(NBLK):
                xb = work.tile([128, 128], BF16, tag="xb")
                nc.sync.dma_start(out=xb[:],
                                  in_=xT_in[:, b * 128:(b + 1) * 128])
                ps = pp.tile([128, 128], F32, tag="mm")
                nc.tensor.matmul(out=ps[:], lhsT=lin1W[:], rhs=xb[:],
                                 start=True, stop=True)
                y = work.tile([128, 128], BF16, tag="y16")
                nc.scalar.activation(out=y[:], in_=ps[:], func=AF.Relu,
                                     bias=lin1b[:], scale=1.0)
                psT = pp.tile([128, 128], F32, tag="mm")
                nc.tensor.matmul(out=psT[:], lhsT=W1, rhs=y[:],
                                 start=True, stop=True)
                nc.vector.tensor_tensor(
                    out=acc[:, b * 128:(b + 1) * 128], in0=psT[:],
                    in1=dinvrep[:, b * 128:(b + 1) * 128], op=AOP.mult)

            nlayers = {"l0g": 1, "l0": 1}.get(cut, LAYERS)
            gather_only = cut == "l0g"

            def musd_from(stg_ap):
                stg16 = work.tile([128, 2], BF16, tag="stg16")
                nc.vector.tensor_copy(out=stg16[:], in_=stg_ap)
                ps_s = pp.tile([1, 2], F32, tag="mm")
                nc.tensor.matmul(out=ps_s[:], lhsT=ones_col[:],
                                 rhs=stg16[:], start=True, stop=True)
                sc = work.tile([1, 4], F32, tag="sc")
                nc.scalar.activation(out=sc[:, 0:2], in_=ps_s[:],
                                     func=AF.Copy, bias=0.0,
                                     scale=1.0 / (N_NODES * F))
                nc.vector.tensor_tensor(out=sc[:, 2:3], in0=sc[:, 0:1],
                                        in1=sc[:, 0:1], op=AOP.mult)
                nc.vector.tensor_tensor(out=sc[:, 2:3], in0=sc[:, 1:2],
                                        in1=sc[:, 2:3], op=AOP.subtract)
                nc.vector.tensor_scalar(out=sc[:, 2:3], in0=sc[:, 2:3],
                                        scalar1=EPS, scalar2=None,
                                        op0=AOP.add)
                nc.vector.reciprocal(out=sc[:, 3:4], in_=sc[:, 2:3])
                nc.scalar.activation(out=sc[:, 3:4], in_=sc[:, 3:4],
                                     func=AF.Sqrt, bias=0.0, scale=1.0)
                sc16 = work.tile([1, 4], BF16, tag="sc16")
                nc.vector.tensor_copy(out=sc16[:], in_=sc[:])
                ps_b = pp.tile([128, 4], F32, tag="mm")
                nc.tensor.matmul(out=ps_b[:], lhsT=ones_row1[:],
                                 rhs=sc16[:], start=True, stop=True)
                musd = work.tile([128, 4], F32, tag="musd")
                nc.vector.tensor_copy(out=musd[:], in_=ps_b[:])
                return musd

            musd_prev = None
            cw_prev = None
            pool_ps = None
            pending_stats = None          # (li_prev, stats tile)
            AR_CH = 96

            for li in range(nlayers):
                parity = li % 2
                stats = work.tile([128, 2], F32, tag="stats")
                nc.vector.memset(stats[:], 0.0)

                # chunk index after which window-3's piece-j blocks are done
                fin_at = {}
                for j in range(NPIECE):
                    last_b = PIECE_BSTART[j] + PIECE_BLKS[j] - 1
                    tidx = np.where((tile_window == NPIECE - 1)
                                    & (tile_block == last_b))[0]
                    fin_at.setdefault(int(tidx.max()) // TPC, []).append(j)

                W_next = (convW[:, (li + 1) * F:(li + 2) * F]
                          if li < LAYERS - 1 else None)
                if li == LAYERS - 1:
                    pool_ps = ppool.tile([128, NGRAPH], F32, tag="pool")

                def finalize_piece(j, li=li, W_next=W_next, stats=stats):
                    sl = slice(j * STRW, (j + 1) * STRW)
                    t1 = escp.tile([128, STRW], F32, tag="esc")
                    nc.vector.tensor_tensor(out=t1[:], in0=acc[:, sl],
                                            in1=dinvrep[:, sl], op=AOP.mult)
                    if li > 0:
                        nc.vector.tensor_scalar(
                            out=t1[:], in0=t1[:],
                            scalar1=musd_prev[:, 3:4], scalar2=None,
                            op0=AOP.mult)
                        t2 = escp.tile([128, STRW], F32, tag="esc")
                        nc.vector.tensor_scalar(
                            out=t2[:], in0=ssrep[:, sl],
                            scalar1=cw_prev[:, 0:1], scalar2=None,
                            op0=AOP.mult)
                        nc.vector.tensor_tensor(out=t1[:], in0=t1[:],
                                                in1=t2[:], op=AOP.subtract)
                    s1 = work.tile([128, 1], F32, tag="s1")
                    nc.scalar.activation(out=acc[:, sl], in_=t1[:],
                                         func=AF.Relu,
                                         bias=convb[:, li:li + 1], scale=1.0,
                                         accum_out=s1[:])
                    sq = escp.tile([128, STRW], F32, tag="esc")
                    s2 = work.tile([128, 1], F32, tag="s2")
                    nc.scalar.activation(out=sq[:], in_=acc[:, sl],
                                         func=AF.Square, bias=0.0, scale=1.0,
                                         accum_out=s2[:])
                    nc.vector.tensor_tensor(out=stats[:, 0:1],
                                            in0=stats[:, 0:1],
                                            in1=s1[:], op=AOP.add)
                    nc.vector.tensor_tensor(out=stats[:, 1:2],
                                            in0=stats[:, 1:2],
                                            in1=s2[:], op=AOP.add)
                    if li < LAYERS - 1:
                        for bl in range(PIECE_BLKS[j]):
                            b = PIECE_BSTART[j] + bl
                            yb = work.tile([128, 128], BF16, tag="y16")
                            nc.vector.tensor_copy(
                                out=yb[:], in_=acc[:, b * 128:(b + 1) * 128])
                            ps = pp.tile([128, F], F32, tag="mm")
                            nc.tensor.matmul(out=ps[:], lhsT=yb[:],
                                             rhs=W_next, start=True,
                                             stop=True)
                            hb = work.tile([128, F], BF16, tag="hb")
                            nc.vector.tensor_scalar(
                                out=hb[:], in0=ps[:],
                                scalar1=dinvw[:, b:b + 1], scalar2=None,
                                op0=AOP.mult)
                            nc.sync.dma_start(out=zin_ap(j, bl), in_=hb[:])
                            psT = pp.tile([128, 128], F32, tag="mm")
                            nc.tensor.matmul(out=psT[:], lhsT=W_next,
                                             rhs=yb[:], start=True, stop=True)
                            nc.vector.tensor_tensor(
                                out=acc[:, b * 128:(b + 1) * 128],
                                in0=psT[:],
                                in1=dinvrep[:, b * 128:(b + 1) * 128],
                                op=AOP.mult)
                        if j % 2 == 1:
                            nc.gpsimd.collective_compute(
                                "AllGather", AOP.bypass, replica_groups=rg,
                                ins=[zin[j // 2][:]],
                                outs=[tab[(li + 1) % 2][j // 2][:]])
                    else:
                        for bl in range(PIECE_BLKS[j]):
                            b = PIECE_BSTART[j] + bl
                            yb = work.tile([128, 128], BF16, tag="y16")
                            nc.vector.tensor_copy(
                                out=yb[:], in_=acc[:, b * 128:(b + 1) * 128])
                            ps_t = pp.tile([128, 128], BF16, tag="mm")
                            nc.tensor.transpose(out=ps_t[:], in_=yb[:],
                                                identity=ident[:])
                            h3 = work.tile([128, 128], BF16, tag="h3")
                            nc.vector.tensor_copy(out=h3[:], in_=ps_t[:])
                            segpz = work.tile([128, NGRAPH], BF16,
                                              tag="segp")
                            nc.vector.tensor_tensor(
                                out=segpz[:],
                                in0=pslot[:, b:b + 1]
                                .to_broadcast([128, NGRAPH]),
                                in1=iota256[:], op=AOP.is_equal)
                            nc.tensor.matmul(out=pool_ps[:], lhsT=h3[:],
                                             rhs=segpz[:],
                                             start=(b == 0),
                                             stop=(b == NBLK - 1),
                                             skip_group_check=True)

                # ---- gather phase with mid-phase finalization ----
                idx_slab = None
                for ch in range(NCHUNK):
                    if ch % SLABCH == 0:
                        idx_slab = ipool.tile(
                            [128, SLABCH * CHUNK // 16], I16, tag="idxslab")
                        wsl = min(SLABCH * CHUNK, ECAP - ch * CHUNK) // 16
                        nc.sync.dma_start(
                            out=idx_slab[:, :wsl],
                            in_=idx_in[:, ch * CHUNK // 16:
                                       ch * CHUNK // 16 + wsl])
                    t0 = ch * TPC
                    w = int(tile_window[t0])
                    gb = gpool.tile([128, TPC, F], BF16, tag="gb")
                    off = (ch % SLABCH) * (CHUNK // 16)
                    nc.gpsimd.dma_gather(
                        gb[:], win_ap(parity, w),
                        idx_slab[:, off:off + CHUNK // 16],
                        CHUNK, CHUNK, F, single_packet=True,
                        queue_num=ch % NQUEUE)
                    gbf = gb[:].rearrange("p a b -> p (a b)")
                    if gather_only:
                        nc.vector.tensor_tensor(
                            out=acc[:, 0:1], in0=acc[:, 0:1],
                            in1=gbf[:, 0:1], op=AOP.add)
                        continue
                    seg = segp.tile([128, TPC, 128], BF16, tag="seg")
                    nc.vector.tensor_tensor(
                        out=seg[:],
                        in0=slot_t[:, t0:t0 + TPC]
                        .rearrange("p (t o) -> p t o", o=1)
                        .to_broadcast([128, TPC, 128]),
                        in1=iota128[:]
                        .rearrange("p (o f) -> p o f", o=1)
                        .to_broadcast([128, TPC, 128]),
                        op=AOP.is_equal)
                    segf = seg[:].rearrange("p a b -> p (a b)")
                    t = 0
                    while t < TPC:
                        b = int(tile_block[t0 + t])
                        t1 = t
                        while t1 < TPC and int(tile_block[t0 + t1]) == b:
                            t1 += 1
                        ps = pag.tile([128, 128], F32, tag="agg")
                        for k in range(t, t1):
                            nc.tensor.matmul(
                                out=ps[:], lhsT=gbf[:, k * F:(k + 1) * F],
                                rhs=segf[:, k * 128:(k + 1) * 128],
                                start=(k == t), stop=(k == t1 - 1))
                        nc.vector.tensor_tensor(
                            out=acc[:, b * 128:(b + 1) * 128],
                            in0=acc[:, b * 128:(b + 1) * 128],
                            in1=ps[:], op=AOP.add)
                        t = t1
                    if ch == AR_CH and pending_stats is not None:
                        li_prev, stats_prev = pending_stats
                        pending_stats = None
                        st_in = dram2.tile([128, 2], F32, tag="stin",
                                           name=f"stin{li_prev}")
                        st_out = dram2.tile([128, 2], F32, tag="stout",
                                            name=f"stout{li_prev}")
                        nc.sync.dma_start(out=st_in[:], in_=stats_prev[:])
                        nc.gpsimd.collective_compute(
                            "AllReduce", AOP.add, replica_groups=rg,
                            ins=[st_in[:]], outs=[st_out[:]])
                        stg = work.tile([128, 2], F32, tag="stg")
                        nc.sync.dma_start(out=stg[:], in_=st_out[:])
                        musd_prev = musd_from(stg[:])
                        cw_prev = work.tile([128, 1], F32, tag="cw")
                        nc.vector.tensor_tensor(
                            out=cw_prev[:],
                            in0=wbar[:, li_prev + 1:li_prev + 2],
                            in1=musd_prev[:, 0:1], op=AOP.mult)
                        nc.vector.tensor_tensor(
                            out=cw_prev[:], in0=cw_prev[:],
                            in1=musd_prev[:, 3:4], op=AOP.mult)
                    if not gather_only:
                        for j in fin_at.get(ch, []):
                            finalize_piece(j)

                if gather_only:
                    break
                if li < LAYERS - 1:
                    pending_stats = (li, stats)

            # ---------------- tail: joint AllReduce + MLP head ----------------
            if pool_ps is None:
                z = work.tile([128, NCLS], F32, tag="zz")
                nc.vector.memset(z[:], 0.0)
                nc.sync.dma_start(out=out_ext[0:128, :], in_=z[:])
                nc.sync.dma_start(out=out_ext[128:256, :], in_=z[:])
            else:
                pooledT = work.tile([128, NGRAPH], F32, tag="pooledT")
                nc.vector.tensor_copy(out=pooledT[:], in_=pool_ps[:])
                cat_in = dram2.tile([128, NGRAPH + 2], F32, tag="catin")
                cat_out = dram2.tile([128, NGRAPH + 2], F32, tag="catout")
                nc.sync.dma_start(out=cat_in[:, 0:NGRAPH], in_=pooledT[:])
                nc.sync.dma_start(out=cat_in[:, NGRAPH:NGRAPH + 2],
                                  in_=stats[:])
                nc.gpsimd.collective_compute(
                    "AllReduce", AOP.add, replica_groups=rg,
                    ins=[cat_in[:]], outs=[cat_out[:]])
                allred = work.tile([128, NGRAPH + 2], F32, tag="allred")
                nc.sync.dma_start(out=allred[:], in_=cat_out[:])
                musd3 = musd_from(allred[:, NGRAPH:NGRAPH + 2])
                invcnt = work.tile([128, NGRAPH], F32, tag="invcnt")
                nc.sync.dma_start(out=invcnt[:], in_=invcntr_in[:])
                pooled = work.tile([128, NGRAPH], F32, tag="pooled2")
                nc.vector.tensor_tensor(out=pooled[:],
                                        in0=allred[:, 0:NGRAPH],
                                        in1=invcnt[:], op=AOP.mult)
                nc.vector.tensor_scalar(out=pooled[:], in0=pooled[:],
                                        scalar1=musd3[:, 0:1],
                                        scalar2=musd3[:, 3:4],
                                        op0=AOP.subtract, op1=AOP.mult)
                pooled16 = work.tile([128, NGRAPH], BF16, tag="pooled16")
                nc.vector.tensor_copy(out=pooled16[:], in_=pooled[:])

                mlpW1 = work.tile([F, F], BF16, tag="mlpW1")
                nc.sync.dma_start(out=mlpW1[:], in_=mlpW1_in[:])
                mlpb1 = work.tile([F, 1], F32, tag="mlpb1")
                nc.sync.dma_start(out=mlpb1[:], in_=mlpb1_in[:])
                mlpW2 = work.tile([F, NCLS], BF16, tag="mlpW2")
                nc.sync.dma_start(out=mlpW2[:], in_=mlpW2_in[:])
                mlpb2r = work.tile([128, NCLS], F32, tag="mlpb2r")
                nc.sync.dma_start(out=mlpb2r[:], in_=mlpb2r_in[:])

                ps_g = pp.tile([128, NGRAPH], F32, tag="mm")
                nc.tensor.matmul(out=ps_g[:], lhsT=mlpW1[:], rhs=pooled16[:],
                                 start=True, stop=True)
                gT = work.tile([128, NGRAPH], BF16, tag="gT")
                nc.scalar.activation(out=gT[:], in_=ps_g[:], func=AF.Relu,
                                     bias=mlpb1[:], scale=1.0)
                for half in range(2):
                    ps_sc = pp.tile([128, NCLS], F32, tag="mm")
                    nc.tensor.matmul(out=ps_sc[:],
                                     lhsT=gT[:, half * 128:(half + 1) * 128],
                                     rhs=mlpW2[:], start=True, stop=True)
                    scr = work.tile([128, NCLS], F32, tag="scr")
                    nc.vector.tensor_tensor(out=scr[:], in0=ps_sc[:],
                                            in1=mlpb2r[:], op=AOP.add)
                    mx = work.tile([128, 1], F32, tag="mx")
                    nc.vector.tensor_reduce(out=mx[:], in_=scr[:],
                                            axis=mybir.AxisListType.X,
                                            op=AOP.max)
                    nc.vector.tensor_scalar(out=scr[:], in0=scr[:],
                                            scalar1=mx[:], scalar2=None,
                                            op0=AOP.subtract)
                    ex = work.tile([128, NCLS], F32, tag="ex")
                    sm = work.tile([128, 1], F32, tag="sm")
                    nc.scalar.activation(out=ex[:], in_=scr[:], func=AF.Exp,
                                         bias=0.0, scale=1.0, accum_out=sm[:])
                    ls = work.tile([128, 1], F32, tag="ls")
                    nc.scalar.activation(out=ls[:], in_=sm[:], func=AF.Ln,
                                         bias=0.0, scale=1.0)
                    nc.vector.tensor_scalar(out=scr[:], in0=scr[:],
                                            scalar1=ls[:], scalar2=None,
                                            op0=AOP.subtract)
                    nc.sync.dma_start(
                        out=out_ext[half * 128:(half + 1) * 128, :],
                        in_=scr[:])

    nc.compile()
    return nc


def _prepare(inputs):
    x = np.asarray(inputs["x"], dtype=np.float32)
    edge_index = np.asarray(inputs["edge_index"])
    batch = np.asarray(inputs["batch"], dtype=np.int64)
    assert x.shape == (N_NODES, F), x.shape

    dinv, S, blk_of, slotin, ls_of, idxw, slotw, meta = \
        _host_preprocess(edge_index)

    cnt = np.bincount(batch, minlength=NGRAPH).astype(np.float64)
    invcnt = (1.0 / np.maximum(cnt, 1.0)).astype(np.float32)
    iota128 = np.broadcast_to(np.arange(128, dtype=np.float32), (128, 128))
    iota256 = np.broadcast_to(np.arange(256, dtype=np.float32), (128, 256))

    lin1_W = np.asarray(inputs["lin1_W"], np.float32)
    lin1_b = np.asarray(inputs["lin1_b"], np.float32)
    conv_W = np.asarray(inputs["conv_W"], np.float32)
    conv_b = np.asarray(inputs["conv_b"], np.float32)
    mlp_W1 = np.asarray(inputs["mlp_W1"], np.float32)
    mlp_b1 = np.asarray(inputs["mlp_b1"], np.float32)
    mlp_W2 = np.asarray(inputs["mlp_W2"], np.float32)
    mlp_b2 = np.asarray(inputs["mlp_b2"], np.float32)

    convW_cat = np.concatenate([conv_W[l] for l in range(LAYERS)], axis=1)
    wbar = np.stack([conv_W[l].sum(axis=0) for l in range(LAYERS)], axis=1)

    SS = dinv * S

    # full-table x^T and dinv (same for every core): column cc*SLOTS + ls
    xTf = np.zeros((F, NCORES * SLOTS), np.float32)
    dinvf_pad = np.zeros(NCORES * SLOTS, np.float32)
    for cc in range(NCORES):
        selc = slice(cc * NSH, (cc + 1) * NSH)
        cols = cc * SLOTS + ls_of[selc]
        xTf[:, cols] = x[selc].T
        dinvf_pad[cols] = dinv[selc]
    xTf = xTf.astype(BF)
    dinvwf = np.ascontiguousarray(
        dinvf_pad.reshape(NCORES * NBLK, 128).T).astype(np.float32)

    in_maps = []
    for c in range(NCORES):
        sel = slice(c * NSH, (c + 1) * NSH)
        ls = ls_of[sel]                       # local slot of each real node
        xT = np.zeros((F, SLOTS), np.float32)
        xT[:, ls] = x[sel].T
        dinv_pad = np.zeros(SLOTS, np.float32)
        dinv_pad[ls] = dinv[sel]
        ss_pad = np.zeros(SLOTS, np.float32)
        ss_pad[ls] = SS[sel]
        pslot_pad = np.full(SLOTS, 300.0, np.float32)
        pslot_pad[ls] = batch[sel].astype(np.float32)
        in_maps.append({
            "xT": xT.astype(BF),
            "xTf": xTf,
            "dinvwf": dinvwf,
            "idx": idxw[c],
            "slot": slotw[c],
            "dinvrep": np.ascontiguousarray(
                np.broadcast_to(dinv_pad.astype(BF), (128, SLOTS))),
            "ssrep": np.ascontiguousarray(
                np.broadcast_to(ss_pad.astype(BF), (128, SLOTS))),
            "dinvw": _wrap_cols(dinv_pad, 0.0),
            "pslot": _wrap_cols(pslot_pad, 300.0).astype(BF),
            "iota128": iota128.astype(BF),
            "iota256": iota256.astype(BF),
            "lin1W": lin1_W.astype(BF),
            "lin1b": np.ascontiguousarray(lin1_b.reshape(F, 1)),
            "convW": convW_cat.astype(BF),
            "convb": np.ascontiguousarray(conv_b.T),
            "wbar": np.ascontiguousarray(wbar),
            "mlpW1": mlp_W1.astype(BF),
            "mlpb1": np.ascontiguousarray(mlp_b1.reshape(F, 1)),
            "mlpW2": mlp_W2.astype(BF),
            "mlpb2r": np.ascontiguousarray(
                np.broadcast_to(mlp_b2, (128, NCLS)).astype(np.float32)),
            "invcntr": np.ascontiguousarray(
                np.broadcast_to(invcnt, (128, NGRAPH))),
        })
    return meta, in_maps


_CACHED = {}


def kernel_run(inputs, trace=False):
    meta, in_maps = _prepare(inputs)
    cut = os.environ.get("K2CUT", "full")
    key = (meta["TT"], cut)
    if key not in _CACHED:
        _CACHED[key] = _build_program(meta, cut=cut)
    nc = _CACHED[key]
    res = run_bass_kernel_spmd(nc, in_maps, core_ids=list(range(NCORES)),
                               trace=trace)
    out = np.asarray(res.results[0]["out"], dtype=np.float32)
    return out, res.exec_time_ns


def kernel(**inputs):
    out, _ = kernel_run(inputs, trace=False)
    return out


# revision 14
# speedup vs baseline: 1.1220x; 1.1220x over previous
"""Trainium2 Bass kernel v2 for the GCN message-passing model (8 NeuronCores).

Key changes vs v1:
- Balanced dst->block assignment (NBLK=102) cuts gather padding ~18%->~2%.
- LayerNorm stats are deferred: the gathered quantity is z = dinv*(y@W)
  (computable before the stats AllReduce); the LN affine correction is
  applied in the next epilogue via the static per-dst sum S[d] and a rank-1
  (Wbar x SS) term.  This removes the stats AllReduce from the critical path.
- The hws AllGather is split into 4 piece collectives; window-j gathers are
  gated only on collective j, so collectives overlap the gather phase.
- Chunk-local psum segments + batched is_equal seg build per chunk.
"""

import os
import sys

sys.path.insert(0, "/opt/trn_rl_repo")

import numpy as np
import ml_dtypes

import concourse.bass as bass
import concourse.bacc as bacc
import concourse.mybir as mybir
import concourse.tile as tile
from concourse.bass_utils import run_bass_kernel_spmd
from concourse.library_config import mlp as gpsimd_mlp_lib
from concourse.masks import make_identity

NCORES = 8
N_NODES = 100_000
F = 128
NCLS = 10
LAYERS = 3
NGRAPH = 256
EPS = 1e-5
NSH = N_NODES // NCORES            # 12500 real nodes per core
NBLK = 102                         # dst blocks of 128 slots per core
SLOTS = NBLK * 128                 # 13056
NPIECE = 4
PIECE_BLKS = [26, 26, 25, 25]
PIECE_CAP = [pb * 128 for pb in PIECE_BLKS]          # 3328,3328,3200,3200
PIECE_BSTART = [0, 26, 52, 77]
PIECE_SSTART = [0, 3328, 6656, 9856]
PIECE_REAL = NSH // NPIECE                            # 3125 real nodes/piece
CHUNK = 1024
TPC = CHUNK // 128
NQUEUE = 4
SLABCH = 16

BF16 = mybir.dt.bfloat16
F32 = mybir.dt.float32
I16 = mybir.dt.int16
AOP = mybir.AluOpType
AF = mybir.ActivationFunctionType
BF = ml_dtypes.bfloat16


def _balance(dw):
    """Assign nodes to blocks per (core, piece), balancing per-window loads.

    dw: [NCORES, PIECE_REAL, 4] per-node window in-degree vectors for one
    piece across all cores.  Returns block index [NCORES, PIECE_REAL] in
    [0, nb) for the given piece size nb."""
    nc_, n, _ = dw.shape

    def run(nb):
        order = np.argsort(-dw.sum(axis=2), axis=1, kind="stable")
        loads = np.zeros((nc_, nb, 4), np.float64)
        sizes = np.zeros((nc_, nb), np.int64)
        assign = np.empty((nc_, n), np.int64)
        ar = np.arange(nc_)
        for r in range(n):
            i = order[:, r]
            dr = dw[ar, i]                       # [nc, 4]
            cand = (loads + dr[:, None, :]).max(axis=2)   # [nc, nb]
            cand[sizes >= 128] = np.inf
            b = np.argmin(cand, axis=1)
            loads[ar, b] += dr
            sizes[ar, b] += 1
            assign[ar, i] = b
        return assign

    return run


def _host_preprocess(edge_index):
    # self-loops are handled locally on-device (z^T acc init), so the edge
    # stream here contains only the real edges; deg/dinv/S keep the loops.
    src = np.asarray(edge_index[0], dtype=np.int64)
    dst = np.asarray(edge_index[1], dtype=np.int64)
    loop = np.arange(N_NODES, dtype=np.int64)
    src_l = np.concatenate([src, loop])
    dst_l = np.concatenate([dst, loop])
    deg = np.bincount(dst_l, minlength=N_NODES).astype(np.float64)
    dinv = (1.0 / np.sqrt(deg)).astype(np.float32)
    S = np.bincount(dst_l, weights=dinv[src_l].astype(np.float64),
                    minlength=N_NODES).astype(np.float32)

    owner = np.arange(N_NODES) // NSH
    ln = np.arange(N_NODES) % NSH
    piece = ln // PIECE_REAL                       # window of this node
    win_e = piece[src]                             # per-edge window

    # per-node window in-degree vectors
    dwin = np.bincount(dst * 4 + win_e, minlength=N_NODES * 4) \
        .reshape(N_NODES, 4).astype(np.float64)

    # balanced block assignment per (core, piece)
    blk_of = np.empty(N_NODES, np.int64)
    for p in range(NPIECE):
        ids = (np.arange(NCORES)[:, None] * NSH + p * PIECE_REAL
               + np.arange(PIECE_REAL)[None, :])          # [8, 3125]
        runner = _balance(dwin[ids])
        assign = runner(PIECE_BLKS[p])                    # [8, 3125]
        blk_of[ids] = PIECE_BSTART[p] + assign

    # repair pass: swap nodes between blocks (same core+piece) until no
    # (core, block, window) cell exceeds 8 tiles (1024 edges)
    pstart = np.array(PIECE_BSTART + [NBLK])
    cnt = np.zeros((NCORES, NBLK, 4), np.int64)
    np.add.at(cnt, (owner, blk_of, piece), dwin.astype(np.int64))
    cap = 8 * 128
    for _ in range(6000):
        over = cnt - cap
        c, b, w = np.unravel_index(np.argmax(over), over.shape)
        if over[c, b, w] <= 0:
            break
        p = int(np.searchsorted(pstart, b, side="right")) - 1
        blo, bhi = pstart[p], pstart[p + 1]
        b2 = blo + int(np.argmin(cnt[c, blo:bhi, w]))
        # node in (c,b) with largest w-degree; partner in (c,b2) with least
        in_b = np.where((owner == c) & (blk_of == b))[0]
        in_b2 = np.where((owner == c) & (blk_of == b2))[0]
        i = in_b[np.argmax(dwin[in_b, w])]
        blk_of[i] = b2
        cnt[c, b] -= dwin[i].astype(np.int64)
        cnt[c, b2] += dwin[i].astype(np.int64)
        if len(in_b2) >= 128:
            j = in_b2[np.argmin(dwin[in_b2, w])]
            blk_of[j] = b
            cnt[c, b2] -= dwin[j].astype(np.int64)
            cnt[c, b] += dwin[j].astype(np.int64)

    # slot within block: order by node id within each (core, block)
    order = np.lexsort((np.arange(N_NODES), blk_of + owner * NBLK))
    key_cb = (owner * NBLK + blk_of)[order]
    slotin = np.empty(N_NODES, np.int64)
    # running index within equal key runs
    change = np.empty(len(key_cb), bool)
    change[0] = True
    change[1:] = key_cb[1:] != key_cb[:-1]
    runstart = np.maximum.accumulate(np.where(change, np.arange(len(key_cb)), 0))
    slotin[order] = np.arange(len(key_cb)) - runstart
    assert slotin.max() < 128
    ls_of = blk_of * 128 + slotin                  # local slot in [0, SLOTS)

    # gather index within the source's window table
    cap_w = np.array(PIECE_CAP)[piece]
    sstart_w = np.array(PIECE_SSTART)[piece]
    widx = owner * cap_w + (ls_of - sstart_w)
    assert widx.min() >= 0 and (widx < 8 * cap_w).all()

    # group edges by (core, block, window)
    key = (owner[dst] * NBLK + blk_of[dst]) * 4 + win_e
    eorder = np.argsort(key, kind="stable")
    key_s = key[eorder]
    ngroups = NCORES * NBLK * 4
    counts = np.bincount(key_s, minlength=ngroups).reshape(NCORES, NBLK, 4)
    starts = np.zeros(ngroups + 1, dtype=np.int64)
    np.cumsum(counts.reshape(-1), out=starts[1:])
    gidx_s = widx[src[eorder]]
    slot_s = slotin[dst[eorder]]

    T = (counts.max(axis=0) + 127) // 128          # [NBLK, 4]
    for w in range(NPIECE):
        T[NBLK - 1, w] += (-int(T[:, w].sum())) % TPC
    TT = int(T.sum())
    ecap = TT * 128

    tile_block = np.empty(TT, dtype=np.int64)
    tile_window = np.empty(TT, dtype=np.int64)
    t0 = 0
    for w in range(NPIECE):
        for b in range(NBLK):
            n = int(T[b, w])
            tile_block[t0:t0 + n] = b
            tile_window[t0:t0 + n] = w
            t0 += n
    assert t0 == TT

    idx16 = np.zeros((NCORES, ecap), dtype=np.int16)
    slots = np.full((NCORES, ecap), 255, dtype=np.float32)
    for c in range(NCORES):
        pos = 0
        for w in range(NPIECE):
            for b in range(NBLK):
                g = (c * NBLK + b) * 4 + w
                s0, s1 = starts[g], starts[g + 1]
                n = s1 - s0
                idx16[c, pos:pos + n] = gidx_s[s0:s1].astype(np.int16)
                slots[c, pos:pos + n] = slot_s[s0:s1].astype(np.float32)
                pos += int(T[b, w]) * 128
        assert pos == ecap

    idxw = idx16.reshape(NCORES, -1, 16).transpose(0, 2, 1)
    idxw = np.ascontiguousarray(np.tile(idxw, (1, 8, 1)))
    slotw = np.ascontiguousarray(
        slots.reshape(NCORES, TT, 128).transpose(0, 2, 1)).astype(BF)

    meta = dict(TT=TT, tile_block=tile_block, tile_window=tile_window)
    return dinv, S, blk_of, slotin, ls_of, idxw, slotw, meta


def _wrap_cols(vec, fill, dtype=np.float32):
    """[SLOTS] -> [128, NBLK] with slot b*128+p at [p, b]."""
    return np.ascontiguousarray(vec.reshape(NBLK, 128).T).astype(dtype)


def _build_program(meta, cut="full"):
    TT = meta["TT"]
    tile_block = meta["tile_block"]
    tile_window = meta["tile_window"]
    ECAP = TT * 128
    # chunks per window
    wtiles = [int((tile_window == w).sum()) for w in range(NPIECE)]
    assert all(t % TPC == 0 for t in wtiles)
    NCHUNK = ECAP // CHUNK

    nc = bacc.Bacc("TRN2", target_bir_lowering=False, debug=False,
                   num_devices=NCORES, num_swdge_queues=NQUEUE)

    xT_in = nc.declare_dram_parameter("xT", [F, SLOTS], BF16, isOutput=False)
    idx_in = nc.declare_dram_parameter("idx", [128, ECAP // 16], I16, isOutput=False)
    slot_in = nc.declare_dram_parameter("slot", [128, TT], BF16, isOutput=False)
    dinvrep_in = nc.declare_dram_parameter("dinvrep", [128, SLOTS], BF16, isOutput=False)
    ssrep_in = nc.declare_dram_parameter("ssrep", [128, SLOTS], BF16, isOutput=False)
    dinvw_in = nc.declare_dram_parameter("dinvw", [128, NBLK], F32, isOutput=False)
    pslot_in = nc.declare_dram_parameter("pslot", [128, NBLK], BF16, isOutput=False)
    iota128_in = nc.declare_dram_parameter("iota128", [128, 128], BF16, isOutput=False)
    iota256_in = nc.declare_dram_parameter("iota256", [128, 256], BF16, isOutput=False)
    lin1W_in = nc.declare_dram_parameter("lin1W", [F, F], BF16, isOutput=False)
    lin1b_in = nc.declare_dram_parameter("lin1b", [F, 1], F32, isOutput=False)
    convW_in = nc.declare_dram_parameter("convW", [F, LAYERS * F], BF16, isOutput=False)
    convb_in = nc.declare_dram_parameter("convb", [F, LAYERS], F32, isOutput=False)
    wbar_in = nc.declare_dram_parameter("wbar", [F, LAYERS], F32, isOutput=False)
    mlpW1_in = nc.declare_dram_parameter("mlpW1", [F, F], BF16, isOutput=False)
    mlpb1_in = nc.declare_dram_parameter("mlpb1", [F, 1], F32, isOutput=False)
    mlpW2_in = nc.declare_dram_parameter("mlpW2", [F, NCLS], BF16, isOutput=False)
    mlpb2r_in = nc.declare_dram_parameter("mlpb2r", [128, NCLS], F32, isOutput=False)
    invcntr_in = nc.declare_dram_parameter("invcntr", [128, NGRAPH], F32, isOutput=False)
    out_ext = nc.declare_dram_parameter("out", [NGRAPH, NCLS], F32, isOutput=True)

    rg = [list(range(NCORES))]
    NSTRIP = 6
    STRW = SLOTS // NSTRIP          # 2176 cols per strip
    assert STRW * NSTRIP == SLOTS

    with tile.TileContext(nc) as tc:
        with tc.tile_pool(name="const", bufs=1) as cst, \
             tc.tile_pool(name="big", bufs=1) as big, \
             tc.tile_pool(name="work", bufs=4) as work, \
             tc.tile_pool(name="esc", bufs=2) as escp, \
             tc.tile_pool(name="gbuf", bufs=8) as gpool, \
             tc.tile_pool(name="idxs", bufs=3) as ipool, \
             tc.tile_pool(name="seg", bufs=6) as segp, \
             tc.tile_pool(name="psum", bufs=4, space="PSUM") as pp, \
             tc.tile_pool(name="ppool", bufs=1, space="PSUM") as ppool, \
             tc.tile_pool(name="dram", bufs=1, space="DRAM") as dram, \
             tc.tile_pool(name="dram2", bufs=1, space="DRAM") as dram2:

            nc.gpsimd.load_library(gpsimd_mlp_lib)

            # ---- persistent SBUF constants ----
            slot_t = cst.tile([128, TT], BF16)
            nc.sync.dma_start(out=slot_t[:], in_=slot_in[:])
            dinvrep = cst.tile([128, SLOTS], BF16)
            nc.sync.dma_start(out=dinvrep[:], in_=dinvrep_in[:])
            ssrep = cst.tile([128, SLOTS], BF16)
            nc.sync.dma_start(out=ssrep[:], in_=ssrep_in[:])
            dinvw = cst.tile([128, NBLK], F32)
            nc.sync.dma_start(out=dinvw[:], in_=dinvw_in[:])
            pslot = cst.tile([128, NBLK], BF16)
            nc.sync.dma_start(out=pslot[:], in_=pslot_in[:])
            iota128 = cst.tile([128, 128], BF16)
            nc.sync.dma_start(out=iota128[:], in_=iota128_in[:])
            iota256 = cst.tile([128, 256], BF16)
            nc.sync.dma_start(out=iota256[:], in_=iota256_in[:])
            lin1W = cst.tile([F, F], BF16)
            nc.sync.dma_start(out=lin1W[:], in_=lin1W_in[:])
            lin1b = cst.tile([F, 1], F32)
            nc.sync.dma_start(out=lin1b[:], in_=lin1b_in[:])
            convW = cst.tile([F, LAYERS * F], BF16)
            nc.sync.dma_start(out=convW[:], in_=convW_in[:])
            convb = cst.tile([F, LAYERS], F32)
            nc.sync.dma_start(out=convb[:], in_=convb_in[:])
            wbar = cst.tile([F, LAYERS], F32)
            nc.sync.dma_start(out=wbar[:], in_=wbar_in[:])
            ones_col = cst.tile([128, 1], BF16)
            nc.vector.memset(ones_col[:], 1.0)
            ones_row1 = cst.tile([1, 128], BF16)
            nc.vector.memset(ones_row1[:], 1.0)
            ident = cst.tile([128, 128], BF16)
            make_identity(nc, ident[:])

            acc = big.tile([128, SLOTS], F32)

            for _ in range(8):
                g0 = gpool.tile([128, TPC, F], BF16, tag="gb")
                nc.vector.memset(g0[:].rearrange("p a b -> p (a b)"), 0.0)

            # DRAM staging: z pieces + double-buffered gather tables
            zin = [dram.tile([PIECE_CAP[j], F], BF16, tag=f"zin{j}")
                   for j in range(NPIECE)]
            tab = [[dram.tile([NCORES * PIECE_CAP[j], F], BF16,
                              tag=f"tab{par}_{j}")
                    for j in range(NPIECE)] for par in range(2)]

            # ---------------- helpers ----------------
            def emit_pass2(li, get_y16):
                """Produce z_{li+1} pieces + their collectives.  li=-1 is P0."""
                parity = (li + 1) % 2
                W = convW[:, (li + 1) * F:(li + 2) * F]
                for j in range(NPIECE):
                    for bl in range(PIECE_BLKS[j]):
                        b = PIECE_BSTART[j] + bl
                        y16 = get_y16(b)
                        ps = pp.tile([128, F], F32, tag="mm")
                        nc.tensor.matmul(out=ps[:], lhsT=y16, rhs=W,
                                         start=True, stop=True)
                        hb = work.tile([128, F], BF16, tag="hb")
                        nc.vector.tensor_scalar(
                            out=hb[:], in0=ps[:],
                            scalar1=dinvw[:, b:b + 1], scalar2=None,
                            op0=AOP.mult)
                        nc.sync.dma_start(out=zin_ap(j, bl), in_=hb[:])
                        psT = pp.tile([128, 128], F32, tag="mm")
                        nc.tensor.matmul(out=psT[:], lhsT=W, rhs=y16,
                                         start=True, stop=True)
                        nc.vector.tensor_tensor(
                            out=acc[:, b * 128:(b + 1) * 128], in0=psT[:],
                            in1=dinvrep[:, b * 128:(b + 1) * 128],
                            op=AOP.mult)
                    if j % 2 == 1:
                        nc.gpsimd.collective_compute(
                            "AllGather", AOP.bypass, replica_groups=rg,
                            ins=[zin[j // 2][:]],
                            outs=[tab[parity][j // 2][:]])

            # ---------------- P0 ----------------
            def p0_y16(b):
                xb = work.tile([128, 128], BF16, tag="xb")
                nc.sync.dma_start(out=xb[:],
                                  in_=xT_in[:, b * 128:(b + 1) * 128])
                ps = pp.tile([128, 128], F32, tag="mm")
                nc.tensor.matmul(out=ps[:], lhsT=lin1W[:], rhs=xb[:],
                                 start=True, stop=True)
                y = work.tile([128, 128], BF16, tag="y16")
                nc.scalar.activation(out=y[:], in_=ps[:], func=AF.Relu,
                                     bias=lin1b[:], scale=1.0)
                return y[:]

            emit_pass2(-1, p0_y16)

            nlayers = {"l0g": 1, "l0": 1}.get(cut, LAYERS)
            gather_only = cut == "l0g"

            def musd_from(stg_ap):
                stg16 = work.tile([128, 2], BF16, tag="stg16")
                nc.vector.tensor_copy(out=stg16[:], in_=stg_ap)
                ps_s = pp.tile([1, 2], F32, tag="mm")
                nc.tensor.matmul(out=ps_s[:], lhsT=ones_col[:],
                                 rhs=stg16[:], start=True, stop=True)
                sc = work.tile([1, 4], F32, tag="sc")
                nc.scalar.activation(out=sc[:, 0:2], in_=ps_s[:],
                                     func=AF.Copy, bias=0.0,
                                     scale=1.0 / (N_NODES * F))
                nc.vector.tensor_tensor(out=sc[:, 2:3], in0=sc[:, 0:1],
                                        in1=sc[:, 0:1], op=AOP.mult)
                nc.vector.tensor_tensor(out=sc[:, 2:3], in0=sc[:, 1:2],
                                        in1=sc[:, 2:3], op=AOP.subtract)
                nc.vector.tensor_scalar(out=sc[:, 2:3], in0=sc[:, 2:3],
                                        scalar1=EPS, scalar2=None,
                                        op0=AOP.add)
                nc.vector.reciprocal(out=sc[:, 3:4], in_=sc[:, 2:3])
                nc.scalar.activation(out=sc[:, 3:4], in_=sc[:, 3:4],
                                     func=AF.Sqrt, bias=0.0, scale=1.0)
                sc16 = work.tile([1, 4], BF16, tag="sc16")
                nc.vector.tensor_copy(out=sc16[:], in_=sc[:])
                ps_b = pp.tile([128, 4], F32, tag="mm")
                nc.tensor.matmul(out=ps_b[:], lhsT=ones_row1[:],
                                 rhs=sc16[:], start=True, stop=True)
                musd = work.tile([128, 4], F32, tag="musd")
                nc.vector.tensor_copy(out=musd[:], in_=ps_b[:])
                return musd

            musd_prev = None
            cw_prev = None
            pool_ps = None
            pending_stats = None          # (li_prev, stats tile)
            AR_CH = 96

            for li in range(nlayers):
                parity = li % 2
                stats = work.tile([128, 2], F32, tag="stats")
                nc.vector.memset(stats[:], 0.0)

                # chunk index after which window-3's piece-j blocks are done
                fin_at = {}
                for j in range(NPIECE):
                    last_b = PIECE_BSTART[j] + PIECE_BLKS[j] - 1
                    tidx = np.where((tile_window == NPIECE - 1)
                                    & (tile_block == last_b))[0]
                    fin_at.setdefault(int(tidx.max()) // TPC, []).append(j)

                W_next = (convW[:, (li + 1) * F:(li + 2) * F]
                          if li < LAYERS - 1 else None)
                if li == LAYERS - 1:
                    pool_ps = ppool.tile([128, NGRAPH], F32, tag="pool")

                def finalize_piece(j, li=li, W_next=W_next, stats=stats):
                    sl = slice(j * STRW, (j + 1) * STRW)
                    t1 = escp.tile([128, STRW], F32, tag="esc")
                    nc.vector.tensor_tensor(out=t1[:], in0=acc[:, sl],
                                            in1=dinvrep[:, sl], op=AOP.mult)
                    if li > 0:
                        nc.vector.tensor_scalar(
                            out=t1[:], in0=t1[:],
                            scalar1=musd_prev[:, 3:4], scalar2=None,
                            op0=AOP.mult)
                        t2 = escp.tile([128, STRW], F32, tag="esc")
                        nc.vector.tensor_scalar(
                            out=t2[:], in0=ssrep[:, sl],
                            scalar1=cw_prev[:, 0:1], scalar2=None,
                            op0=AOP.mult)
                        nc.vector.tensor_tensor(out=t1[:], in0=t1[:],
                                                in1=t2[:], op=AOP.subtract)
                    s1 = work.tile([128, 1], F32, tag="s1")
                    nc.scalar.activation(out=acc[:, sl], in_=t1[:],
                                         func=AF.Relu,
                                         bias=convb[:, li:li + 1], scale=1.0,
                                         accum_out=s1[:])
                    sq = escp.tile([128, STRW], F32, tag="esc")
                    s2 = work.tile([128, 1], F32, tag="s2")
                    nc.scalar.activation(out=sq[:], in_=acc[:, sl],
                                         func=AF.Square, bias=0.0, scale=1.0,
                                         accum_out=s2[:])
                    nc.vector.tensor_tensor(out=stats[:, 0:1],
                                            in0=stats[:, 0:1],
                                            in1=s1[:], op=AOP.add)
                    nc.vector.tensor_tensor(out=stats[:, 1:2],
                                            in0=stats[:, 1:2],
                                            in1=s2[:], op=AOP.add)
                    if li < LAYERS - 1:
                        for bl in range(PIECE_BLKS[j]):
                            b = PIECE_BSTART[j] + bl
                            yb = work.tile([128, 128], BF16, tag="y16")
                            nc.vector.tensor_copy(
                                out=yb[:], in_=acc[:, b * 128:(b + 1) * 128])
                            ps = pp.tile([128, F], F32, tag="mm")
                            nc.tensor.matmul(out=ps[:], lhsT=yb[:],
                                             rhs=W_next, start=True,
                                             stop=True)
                            hb = work.tile([128, F], BF16, tag="hb")
                            nc.vector.tensor_scalar(
                                out=hb[:], in0=ps[:],
                                scalar1=dinvw[:, b:b + 1], scalar2=None,
                                op0=AOP.mult)
                            nc.sync.dma_start(out=zin_ap(j, bl), in_=hb[:])
                            psT = pp.tile([128, 128], F32, tag="mm")
                            nc.tensor.matmul(out=psT[:], lhsT=W_next,
                                             rhs=yb[:], start=True, stop=True)
                            nc.vector.tensor_tensor(
                                out=acc[:, b * 128:(b + 1) * 128],
                                in0=psT[:],
                                in1=dinvrep[:, b * 128:(b + 1) * 128],
                                op=AOP.mult)
                        if j % 2 == 1:
                            nc.gpsimd.collective_compute(
                                "AllGather", AOP.bypass, replica_groups=rg,
                                ins=[zin[j // 2][:]],
                                outs=[tab[(li + 1) % 2][j // 2][:]])
                    else:
                        for bl in range(PIECE_BLKS[j]):
                            b = PIECE_BSTART[j] + bl
                            yb = work.tile([128, 128], BF16, tag="y16")
                            nc.vector.tensor_copy(
                                out=yb[:], in_=acc[:, b * 128:(b + 1) * 128])
                            ps_t = pp.tile([128, 128], BF16, tag="mm")
                            nc.tensor.transpose(out=ps_t[:], in_=yb[:],
                                                identity=ident[:])
                            h3 = work.tile([128, 128], BF16, tag="h3")
                            nc.vector.tensor_copy(out=h3[:], in_=ps_t[:])
                            segpz = work.tile([128, NGRAPH], BF16,
                                              tag="segp")
                            nc.vector.tensor_tensor(
                                out=segpz[:],
                                in0=pslot[:, b:b + 1]
                                .to_broadcast([128, NGRAPH]),
                                in1=iota256[:], op=AOP.is_equal)
                            nc.tensor.matmul(out=pool_ps[:], lhsT=h3[:],
                                             rhs=segpz[:],
                                             start=(b == 0),
                                             stop=(b == NBLK - 1),
                                             skip_group_check=True)

                # ---- gather phase with mid-phase finalization ----
                idx_slab = None
                for ch in range(NCHUNK):
                    if ch % SLABCH == 0:
                        idx_slab = ipool.tile(
                            [128, SLABCH * CHUNK // 16], I16, tag="idxslab")
                        wsl = min(SLABCH * CHUNK, ECAP - ch * CHUNK) // 16
                        nc.sync.dma_start(
                            out=idx_slab[:, :wsl],
                            in_=idx_in[:, ch * CHUNK // 16:
                                       ch * CHUNK // 16 + wsl])
                    t0 = ch * TPC
                    w = int(tile_window[t0])
                    gb = gpool.tile([128, TPC, F], BF16, tag="gb")
                    off = (ch % SLABCH) * (CHUNK // 16)
                    nc.gpsimd.dma_gather(
                        gb[:], win_ap(parity, w),
                        idx_slab[:, off:off + CHUNK // 16],
                        CHUNK, CHUNK, F, single_packet=True,
                        queue_num=ch % NQUEUE)
                    gbf = gb[:].rearrange("p a b -> p (a b)")
                    if gather_only:
                        nc.vector.tensor_tensor(
                            out=acc[:, 0:1], in0=acc[:, 0:1],
                            in1=gbf[:, 0:1], op=AOP.add)
                        continue
                    seg = segp.tile([128, TPC, 128], BF16, tag="seg")
                    nc.vector.tensor_tensor(
                        out=seg[:],
                        in0=slot_t[:, t0:t0 + TPC]
                        .rearrange("p (t o) -> p t o", o=1)
                        .to_broadcast([128, TPC, 128]),
                        in1=iota128[:]
                        .rearrange("p (o f) -> p o f", o=1)
                        .to_broadcast([128, TPC, 128]),
                        op=AOP.is_equal)
                    segf = seg[:].rearrange("p a b -> p (a b)")
                    t = 0
                    while t < TPC:
                        b = int(tile_block[t0 + t])
                        t1 = t
                        while t1 < TPC and int(tile_block[t0 + t1]) == b:
                            t1 += 1
                        ps = pag.tile([128, 128], F32, tag="agg")
                        for k in range(t, t1):
                            nc.tensor.matmul(
                                out=ps[:], lhsT=gbf[:, k * F:(k + 1) * F],
                                rhs=segf[:, k * 128:(k + 1) * 128],
                                start=(k == t), stop=(k == t1 - 1))
                        nc.vector.tensor_tensor(
                            out=acc[:, b * 128:(b + 1) * 128],
                            in0=acc[:, b * 128:(b + 1) * 128],
                            in1=ps[:], op=AOP.add)
                        t = t1
                    if ch == AR_CH and pending_stats is not None:
                        li_prev, stats_prev = pending_stats
                        pending_stats = None
                        st_in = dram2.tile([128, 2], F32, tag="stin",
                                           name=f"stin{li_prev}")
                        st_out = dram2.tile([128, 2], F32, tag="stout",
                                            name=f"stout{li_prev}")
                        nc.sync.dma_start(out=st_in[:], in_=stats_prev[:])
                        nc.gpsimd.collective_compute(
                            "AllReduce", AOP.add, replica_groups=rg,
                            ins=[st_in[:]], outs=[st_out[:]])
                        stg = work.tile([128, 2], F32, tag="stg")
                        nc.sync.dma_start(out=stg[:], in_=st_out[:])
                        musd_prev = musd_from(stg[:])
                        cw_prev = work.tile([128, 1], F32, tag="cw")
                        nc.vector.tensor_tensor(
                            out=cw_prev[:],
                            in0=wbar[:, li_prev + 1:li_prev + 2],
                            in1=musd_prev[:, 0:1], op=AOP.mult)
                        nc.vector.tensor_tensor(
                            out=cw_prev[:], in0=cw_prev[:],
                            in1=musd_prev[:, 3:4], op=AOP.mult)
                    if not gather_only:
                        for j in fin_at.get(ch, []):
                            finalize_piece(j)

                if gather_only:
                    break
                if li < LAYERS - 1:
                    pending_stats = (li, stats)

            # ---------------- tail: joint AllReduce + MLP head ----------------
            if pool_ps is None:
                z = work.tile([128, NCLS], F32, tag="zz")
                nc.vector.memset(z[:], 0.0)
                nc.sync.dma_start(out=out_ext[0:128, :], in_=z[:])
                nc.sync.dma_start(out=out_ext[128:256, :], in_=z[:])
            else:
                pooledT = work.tile([128, NGRAPH], F32, tag="pooledT")
                nc.vector.tensor_copy(out=pooledT[:], in_=pool_ps[:])
                cat_in = dram2.tile([128, NGRAPH + 2], F32, tag="catin")
                cat_out = dram2.tile([128, NGRAPH + 2], F32, tag="catout")
                nc.sync.dma_start(out=cat_in[:, 0:NGRAPH], in_=pooledT[:])
                nc.sync.dma_start(out=cat_in[:, NGRAPH:NGRAPH + 2],
                                  in_=stats[:])
                nc.gpsimd.collective_compute(
                    "AllReduce", AOP.add, replica_groups=rg,
                    ins=[cat_in[:]], outs=[cat_out[:]])
                allred = work.tile([128, NGRAPH + 2], F32, tag="allred")
                nc.sync.dma_start(out=allred[:], in_=cat_out[:])
                musd3 = musd_from(allred[:, NGRAPH:NGRAPH + 2])
                invcnt = work.tile([128, NGRAPH], F32, tag="invcnt")
                nc.sync.dma_start(out=invcnt[:], in_=invcntr_in[:])
                pooled = work.tile([128, NGRAPH], F32, tag="pooled2")
                nc.vector.tensor_tensor(out=pooled[:],
                                        in0=allred[:, 0:NGRAPH],
                                        in1=invcnt[:], op=AOP.mult)
                nc.vector.tensor_scalar(out=pooled[:], in0=pooled[:],
                                        scalar1=musd3[:, 0:1],
                                        scalar2=musd3[:, 3:4],
                                        op0=AOP.subtract, op1=AOP.mult)
                pooled16 = work.tile([128, NGRAPH], BF16, tag="pooled16")
                nc.vector.tensor_copy(out=pooled16[:], in_=pooled[:])

                mlpW1 = work.tile([F, F], BF16, tag="mlpW1")
                nc.sync.dma_start(out=mlpW1[:], in_=mlpW1_in[:])
                mlpb1 = work.tile([F, 1], F32, tag="mlpb1")
                nc.sync.dma_start(out=mlpb1[:], in_=mlpb1_in[:])
                mlpW2 = work.tile([F, NCLS], BF16, tag="mlpW2")
                nc.sync.dma_start(out=mlpW2[:], in_=mlpW2_in[:])
                mlpb2r = work.tile([128, NCLS], F32, tag="mlpb2r")
                nc.sync.dma_start(out=mlpb2r[:], in_=mlpb2r_in[:])

                ps_g = pp.tile([128, NGRAPH], F32, tag="mm")
                nc.tensor.matmul(out=ps_g[:], lhsT=mlpW1[:], rhs=pooled16[:],
                                 start=True, stop=True)
                gT = work.tile([128, NGRAPH], BF16, tag="gT")
                nc.scalar.activation(out=gT[:], in_=ps_g[:], func=AF.Relu,
                                     bias=mlpb1[:], scale=1.0)
                for half in range(2):
                    ps_sc = pp.tile([128, NCLS], F32, tag="mm")
                    nc.tensor.matmul(out=ps_sc[:],
                                     lhsT=gT[:, half * 128:(half + 1) * 128],
                                     rhs=mlpW2[:], start=True, stop=True)
                    scr = work.tile([128, NCLS], F32, tag="scr")
                    nc.vector.tensor_tensor(out=scr[:], in0=ps_sc[:],
                                            in1=mlpb2r[:], op=AOP.add)
                    mx = work.tile([128, 1], F32, tag="mx")
                    nc.vector.tensor_reduce(out=mx[:], in_=scr[:],
                                            axis=mybir.AxisListType.X,
                                            op=AOP.max)
                    nc.vector.tensor_scalar(out=scr[:], in0=scr[:],
                                            scalar1=mx[:], scalar2=None,
                                            op0=AOP.subtract)
                    ex = work.tile([128, NCLS], F32, tag="ex")
                    sm = work.tile([128, 1], F32, tag="sm")
                    nc.scalar.activation(out=ex[:], in_=scr[:], func=AF.Exp,
                                         bias=0.0, scale=1.0, accum_out=sm[:])
                    ls = work.tile([128, 1], F32, tag="ls")
                    nc.scalar.activation(out=ls[:], in_=sm[:], func=AF.Ln,
                                         bias=0.0, scale=1.0)
                    nc.vector.tensor_scalar(out=scr[:], in0=scr[:],
                                            scalar1=ls[:], scalar2=None,
                                            op0=AOP.subtract)
                    nc.sync.dma_start(
                        out=out_ext[half * 128:(half + 1) * 128, :],
                        in_=scr[:])

    nc.compile()
    return nc


def _prepare(inputs):
    x = np.asarray(inputs["x"], dtype=np.float32)
    edge_index = np.asarray(inputs["edge_index"])
    batch = np.asarray(inputs["batch"], dtype=np.int64)
    assert x.shape == (N_NODES, F), x.shape

    dinv, S, blk_of, slotin, ls_of, idxw, slotw, meta = \
        _host_preprocess(edge_index)

    cnt = np.bincount(batch, minlength=NGRAPH).astype(np.float64)
    invcnt = (1.0 / np.maximum(cnt, 1.0)).astype(np.float32)
    iota128 = np.broadcast_to(np.arange(128, dtype=np.float32), (128, 128))
    iota256 = np.broadcast_to(np.arange(256, dtype=np.float32), (128, 256))

    lin1_W = np.asarray(inputs["lin1_W"], np.float32)
    lin1_b = np.asarray(inputs["lin1_b"], np.float32)
    conv_W = np.asarray(inputs["conv_W"], np.float32)
    conv_b = np.asarray(inputs["conv_b"], np.float32)
    mlp_W1 = np.asarray(inputs["mlp_W1"], np.float32)
    mlp_b1 = np.asarray(inputs["mlp_b1"], np.float32)
    mlp_W2 = np.asarray(inputs["mlp_W2"], np.float32)
    mlp_b2 = np.asarray(inputs["mlp_b2"], np.float32)

    convW_cat = np.concatenate([conv_W[l] for l in range(LAYERS)], axis=1)
    wbar = np.stack([conv_W[l].sum(axis=0) for l in range(LAYERS)], axis=1)

    SS = dinv * S

    in_maps = []
    for c in range(NCORES):
        sel = slice(c * NSH, (c + 1) * NSH)
        ls = ls_of[sel]                       # local slot of each real node
        xT = np.zeros((F, SLOTS), np.float32)
        xT[:, ls] = x[sel].T
        dinv_pad = np.zeros(SLOTS, np.float32)
        dinv_pad[ls] = dinv[sel]
        ss_pad = np.zeros(SLOTS, np.float32)
        ss_pad[ls] = SS[sel]
        pslot_pad = np.full(SLOTS, 300.0, np.float32)
        pslot_pad[ls] = batch[sel].astype(np.float32)
        in_maps.append({
            "xT": xT.astype(BF),
            "idx": idxw[c],
            "slot": slotw[c],
            "dinvrep": np.ascontiguousarray(
                np.broadcast_to(dinv_pad.astype(BF), (128, SLOTS))),
            "ssrep": np.ascontiguousarray(
                np.broadcast_to(ss_pad.astype(BF), (128, SLOTS))),
            "dinvw": _wrap_cols(dinv_pad, 0.0),
            "pslot": _wrap_cols(pslot_pad, 300.0).astype(BF),
            "iota128": iota128.astype(BF),
            "iota256": iota256.astype(BF),
            "lin1W": lin1_W.astype(BF),
            "lin1b": np.ascontiguousarray(lin1_b.reshape(F, 1)),
            "convW": convW_cat.astype(BF),
            "convb": np.ascontiguousarray(conv_b.T),
            "wbar": np.ascontiguousarray(wbar),
            "mlpW1": mlp_W1.astype(BF),
            "mlpb1": np.ascontiguousarray(mlp_b1.reshape(F, 1)),
            "mlpW2": mlp_W2.astype(BF),
            "mlpb2r": np.ascontiguousarray(
                np.broadcast_to(mlp_b2, (128, NCLS)).astype(np.float32)),
            "invcntr": np.ascontiguousarray(
                np.broadcast_to(invcnt, (128, NGRAPH))),
        })
    return meta, in_maps


_CACHED = {}


def kernel_run(inputs, trace=False):
    meta, in_maps = _prepare(inputs)
    cut = os.environ.get("K2CUT", "full")
    key = (meta["TT"], cut)
    if key not in _CACHED:
        _CACHED[key] = _build_program(meta, cut=cut)
    nc = _CACHED[key]
    res = run_bass_kernel_spmd(nc, in_maps, core_ids=list(range(NCORES)),
                               trace=trace)
    out = np.asarray(res.results[0]["out"], dtype=np.float32)
    return out, res.exec_time_ns


def kernel(**inputs):
    out, _ = kernel_run(inputs, trace=False)
    return out
